# revision 21
# baseline (speedup 1.0000x reference)
"""MixGARCH Trainium2 kernel.

Reference semantics: scan over t of
    v_t = relu(bias + Wx @ o_t^2 + Wh * v_{t-1}) + 1e-6,  hist[t] = v_t
with bias, Wx, Wh, o^2, v0 all >= 0, so relu is an identity and this is a
LINEAR first-order recurrence:
    v_t = Wh * v_{t-1} + c_t,   c_t = (bias + 1e-6) + Wx @ o_t^2

Strategy (8 cores, full I/O):
 - Each core owns 65536 timesteps, split into 2 halves of 32768. Each half is
   an independent scan lane group (64 components), giving 128 SBUF partitions
   of independent recurrences per core.
 - Cross-boundary state is handled with a 1024-step warmup (Wh < 0.9, so the
   influence of the unknown incoming state decays below fp32 resolution in
   <600 steps; 0.9^1024 ~ 1e-47 == 0.0f). Core 0 half A starts from the exact
   v0 instead (no warmup).
 - On device: PE matmuls compute Wx @ o^2 (zero-padded 32-row weight variants,
   so every matmul is 32-partition aligned), ACT squares the input, copies
   PSUM->SBUF adding (bias + 1e-6) per partition, and DVE tensor_tensor_scan
   runs the recurrence 128 lanes at a time, chained across 512-wide tiles.
 - Host packs the input into the exact SBUF layout (128 = 16 chunks x 8
   channels) and de-interleaves the (128, T'') output back to (T, 64).
"""

import os
import numpy as np

T = 524288
K = 64
NJ = 8
NCORES = 8
W = 1024              # warmup steps per half
HALF = 32768          # real steps per half
TT = W + HALF         # 33792 = per-half scan length
NCH = 8               # chunks per half
CHUNK = TT // NCH     # 4224 elements per partition
F = 512               # scan tile width
NTILES = TT // F      # 66
STAGE = 8 * F         # 4096-wide output staging
MM_DT = os.environ.get("MIXGARCH_MM_DTYPE", "float32")

_CACHE = {}


def _build_nc():
    import concourse.bacc as bacc
    import concourse.mybir as mybir
    import concourse.tile as tile

    mm_dt = getattr(mybir.dt, MM_DT)
    f32 = mybir.dt.float32
    PSUM_BUFS = 6
    CSB_BUFS = 8

    nc = bacc.Bacc(None, target_bir_lowering=False)
    xin = nc.dram_tensor("xin", [128, CHUNK], f32, kind="ExternalInput")
    wt = nc.dram_tensor("wt", [128, 256], f32, kind="ExternalInput")
    biast = nc.dram_tensor("biast", [128, 1], f32, kind="ExternalInput")
    wscan = nc.dram_tensor("wscan", [128, F], f32, kind="ExternalInput")
    vinit = nc.dram_tensor("vinit", [128, 1], f32, kind="ExternalInput")
    vout = nc.dram_tensor("vout", [128, TT], f32, kind="ExternalOutput")

    with tile.TileContext(nc) as tc:
        with (
            tc.tile_pool(name="const", bufs=1) as cpool,
            tc.tile_pool(name="xbuf", bufs=1) as xpool,
            tc.tile_pool(name="cbuf", bufs=1) as cbuf,
            tc.tile_pool(name="stage", bufs=2) as stpool,
            tc.tile_pool(name="psum", bufs=1, space="PSUM") as ps,
        ):
            wt_sb = cpool.tile([128, 256], f32)
            nc.sync.dma_start(wt_sb[:], wt[:])
            bias_sb = cpool.tile([128, 1], f32)
            nc.sync.dma_start(bias_sb[:], biast[:])
            ws_sb = cpool.tile([128, F], f32)
            nc.sync.dma_start(ws_sb[:], wscan[:])
            vi_sb = cpool.tile([128, 1], f32)
            nc.sync.dma_start(vi_sb[:], vinit[:])

            x_sb = xpool.tile([128, CHUNK], f32)
            x2_sb = xpool.tile([128, CHUNK], f32)
            NLOAD = 4
            lw = CHUNK // NLOAD  # 1056
            for q in range(NLOAD):
                sl = slice(q * lw, (q + 1) * lw)
                nc.sync.dma_start(x_sb[:, sl], xin[:, sl])
                nc.scalar.activation(
                    x2_sb[:, sl], x_sb[:, sl], mybir.ActivationFunctionType.Square
                )

            prev_stage = None
            stage_t = None
            for i in range(NTILES):
                slot = i % 8
                if slot == 0:
                    prev_stage = stage_t
                    nst = STAGE if (NTILES - i) >= 8 else (NTILES - i) * F
                    stage_t = stpool.tile([128, nst], f32, tag="stage")

                # Per-slot tags pin PSUM reuse to exactly i - PSUM_BUFS.
                c_ps = ps.tile([128, F], f32, tag=f"cps{i % PSUM_BUFS}")
                for h in range(2):
                    done = 0
                    while done < F:
                        pos = i * F + done
                        c = pos // CHUNK
                        off = pos % CHUNK
                        n = min(F - done, CHUNK - off)
                        g = h * NCH + c
                        b, r = g // 4, g % 4
                        lhsT = wt_sb[32 * b:32 * b + 32, 64 * r:64 * r + 64]
                        rhs = x2_sb[32 * b:32 * b + 32, off:off + n]
                        if MM_DT != "float32":
                            lhsT = lhsT.bitcast(mm_dt)
                            rhs = rhs.bitcast(mm_dt)
                        nc.tensor.matmul(
                            c_ps[64 * h:64 * h + 64, done:done + n],
                            lhsT,
                            rhs,
                            start=True,
                            stop=True,
                            tile_position=(32 * b, 64 * h),
                        )
                        done += n

                c_sb = cbuf.tile([128, F], f32, tag=f"csb{i % CSB_BUFS}")
                nc.scalar.activation(
                    c_sb[:], c_ps[:], mybir.ActivationFunctionType.Identity,
                    bias=bias_sb[:, 0:1],
                )

                initial = (
                    vi_sb[:, 0:1]
                    if i == 0
                    else (
                        stage_t[:, slot * F - 1:slot * F]
                        if slot > 0
                        else prev_stage[:, prev_stage.shape[1] - 1:prev_stage.shape[1]]
                    )
                )
                nc.vector.tensor_tensor_scan(
                    stage_t[:, slot * F:(slot + 1) * F],
                    ws_sb[:],
                    c_sb[:],
                    initial,
                    mybir.AluOpType.mult,
                    mybir.AluOpType.add,
                )

                if slot == 7 or i == NTILES - 1:
                    base = (i - slot) * F
                    nc.sync.dma_start(
                        vout[:, base:base + stage_t.shape[1]], stage_t[:]
                    )

    nc.compile()
    return nc


def _host_prep(series, vars0, bias, Wx, Wh):
    series = np.asarray(series, dtype=np.float32)
    vars0 = np.asarray(vars0, dtype=np.float32)
    bias = np.asarray(bias, dtype=np.float32)
    Wx = np.asarray(Wx, dtype=np.float32)
    Wh = np.asarray(Wh, dtype=np.float32)

    in_maps = []
    wt = np.zeros((128, 256), dtype=np.float32)
    for q in range(4):
        for r in range(4):
            for j in range(NJ):
                wt[32 * q + 8 * r + j, 64 * r:64 * r + 64] = Wx[:, j]
    biasv = np.zeros((128, 1), dtype=np.float32)
    biasv[0:64, 0] = bias + 1e-6
    biasv[64:128, 0] = bias + 1e-6
    wscan = np.zeros((128, F), dtype=np.float32)
    wscan[0:64, :] = Wh[:, None]
    wscan[64:128, :] = Wh[:, None]

    for i in range(NCORES):
        xin = np.empty((128, CHUNK), dtype=np.float32)
        for h in range(2):
            start = i * 65536 + h * HALF
            if i == 0 and h == 0:
                rows = series[0:TT]
            else:
                rows = series[start - W:start + HALF]
            for c in range(NCH):
                g = h * NCH + c
                b, r = g // 4, g % 4
                xin[32 * b + 8 * r:32 * b + 8 * r + 8, :] = (
                    rows[c * CHUNK:(c + 1) * CHUNK, :].T
                )
        vinit = np.zeros((128, 1), dtype=np.float32)
        if i == 0:
            vinit[0:64, 0] = vars0
        in_maps.append(
            {"xin": xin, "wt": wt, "biast": biasv, "wscan": wscan, "vinit": vinit}
        )
    return in_maps


def _assemble(results):
    hist = np.empty((T, K), dtype=np.float32)
    for i in range(NCORES):
        vout = results[i]["vout"]
        for h in range(2):
            start = i * 65536 + h * HALF
            q0 = 0 if (i == 0 and h == 0) else W
            hist[start:start + HALF, :] = vout[64 * h:64 * h + 64,
                                               q0:q0 + HALF].T
    return hist


def run(inputs, trace=False, **kw):
    from concourse.bass_utils import run_bass_kernel_spmd

    if "nc" not in _CACHE:
        _CACHE["nc"] = _build_nc()
    nc = _CACHE["nc"]
    in_maps = _host_prep(
        inputs["series"], inputs["vars0"], inputs["bias"],
        inputs["Wx"], inputs["Wh"],
    )
    res = run_bass_kernel_spmd(
        nc, in_maps, core_ids=list(range(NCORES)), trace=trace, **kw
    )
    return _assemble(res.results), res


def kernel(series, vars0, bias, Wx, Wh):
    out, _ = run(
        {"series": series, "vars0": vars0, "bias": bias, "Wx": Wx, "Wh": Wh}
    )
    return out
